# revision 38
# baseline (speedup 1.0000x reference)
"""GAT (2-layer graph attention network) on 8 Trainium2 NeuronCores.

Strategy: node partition. Core c owns nodes [c*6250, (c+1)*6250) and all edges
whose src lies in its range (segment sums in the reference are over src).
Host-side preprocessing (index manipulation only): sort edges by src, group by
128-node src tile; within each tile order edges as [dst<32768 block | dst>=32768
block], each block sorted by dst and padded to 128-edge chunks. Per layer each
core computes node features for its own nodes (dense matmuls), AllGathers a
compact f16 table, expands it to 256B rows, then runs the sparse phase: batched
dma_gather of table rows by dst (int16 indices, low/high table halves),
attention coefficients on the scalar engine, and segment sums as mask-matmuls
(one-hot src masks built with is_equal vs an iota row) accumulated in PSUM.

Self-contained: only needs numpy + the concourse (Bass) stack at
/opt/trn_rl_repo. All shapes hardcoded for the nn_GAT problem.
"""
import sys

if "/opt/trn_rl_repo" not in sys.path:
    sys.path.insert(0, "/opt/trn_rl_repo")

import numpy as np

import concourse.bacc as bacc
import concourse.bass as bass
import concourse.mybir as mybir
import concourse.tile as tile
from concourse import library_config
from concourse.bass_utils import run_bass_kernel_spmd
from concourse.masks import make_identity

# problem shapes
N = 50000
E = 800000
FIN = 256
H = 8          # heads, layer 1
F1 = 32        # per-head features, layer 1
NH = 256       # hidden = H*F1
C = 47         # classes
NCORES = 8
NPC = N // NCORES          # nodes per core = 6250
T = (NPC + 127) // 128     # src tiles per core = 49
LAST_ROWS = NPC - (T - 1) * 128   # rows in last tile = 106
NPAD = T * 128             # padded node count per core = 6272
SPLIT = 32768              # table row split for int16 gather indices
ROW = 128                  # f16 elements per gather-table row (256B)
NT = (N + 127) // 128      # 128-row tiles covering the full table = 391
MAXI = 896                 # max idxs per single_packet dma_gather call

F32 = mybir.dt.float32
F16 = mybir.dt.float16
I16 = mybir.dt.int16
I8 = mybir.dt.int8

ALU = mybir.AluOpType
ACT = mybir.ActivationFunctionType

_cache = {}


def _wrap_idx(idx):
    """[num] int -> [128, num//16] int16 wrapped in 16 partitions, replicated
    across the 8 gpsimd cores."""
    num = len(idx)
    w = np.empty((128, num // 16), dtype=np.int16)
    blk = idx.astype(np.int16).reshape(num // 16, 16).T
    for g in range(8):
        w[g * 16:(g + 1) * 16, :] = blk
    return w


def _preprocess(x, edge_src, edge_dst):
    """Sort/group/pad edges; build per-core input arrays (numpy, index work
    only). Returns per-core dicts + the per-tile chunk structure."""
    order = np.argsort(edge_src, kind="stable")
    src_s = np.asarray(edge_src)[order].astype(np.int64)
    dst_s = np.asarray(edge_dst)[order].astype(np.int64)
    bounds = np.searchsorted(src_s, np.arange(NCORES + 1) * NPC)

    # per (core, tile): low/high dst blocks, each sorted by dst
    percore = []   # [core][tile] -> (low_dst, high_dst, low_src, high_src)
    KL = np.zeros(T, dtype=np.int64)
    KH = np.zeros(T, dtype=np.int64)
    for c in range(NCORES):
        b0, b1 = bounds[c], bounds[c + 1]
        sc = (src_s[b0:b1] - c * NPC)
        dc = dst_s[b0:b1]
        tloc = sc >> 7
        tiles = []
        for t in range(T):
            m = tloc == t
            st, dt_ = sc[m], dc[m]
            lo = dt_ < SPLIT
            ol = np.argsort(dt_[lo], kind="stable")
            oh = np.argsort(dt_[~lo], kind="stable")
            tiles.append((dt_[lo][ol], dt_[~lo][oh] - SPLIT,
                          st[lo][ol] - t * 128, st[~lo][oh] - t * 128))
            KL[t] = max(KL[t], (len(ol) + 127) // 128)
            KH[t] = max(KH[t], (len(oh) + 127) // 128)
        percore.append(tiles)
    KL = np.maximum(KL, 1)
    KH = np.maximum(KH, 1)
    KTOT = KL + KH
    cpt = int(KTOT.max())          # SBUF tiles sized for the largest tile
    # column offsets of each tile's idx block in the packed idx tensor
    S = (KTOT * 8).astype(np.int64)           # int16 cols per tile (=K*128/16)
    soff = np.concatenate([[0], np.cumsum(S)])
    stot = int(soff[-1])

    # packed per-tile payload: kt cols of srcloc (f16 bits) + kt*8 idx cols
    S9 = (KTOT * 9).astype(np.int64)
    soff9 = np.concatenate([[0], np.cumsum(S9)])

    ins = []
    for c in range(NCORES):
        srcloc = np.full((T, 128, cpt), 1000.0, dtype=np.float32)
        slidx = np.zeros((128, int(soff9[-1])), dtype=np.int16)
        for t in range(T):
            dl, dh, sl_, sh_ = percore[c][t]
            nl, nh_ = len(dl), len(dh)
            kl, kh = int(KL[t]), int(KH[t])
            kt = kl + kh
            # slot j: low block j in [0, kl*128), high block j - kl*128
            il = np.zeros(kl * 128, dtype=np.int64)
            il[:nl] = dl
            ih = np.zeros(kh * 128, dtype=np.int64)
            ih[:nh_] = dh
            j = np.arange(nl)
            srcloc[t, j % 128, j // 128] = sl_
            j = kl * 128 + np.arange(nh_)
            srcloc[t, j % 128, j // 128] = sh_
            o = int(soff9[t])
            slidx[:, o:o + kt] = srcloc[t, :, 0:kt].astype(
                np.float16).view(np.int16)
            slidx[:, o + kt:o + 9 * kt] = _wrap_idx(np.concatenate([il, ih]))
        # edge-major srcflat, pre-broadcast to all 128 partitions so the
        # device reads it as a plain (parallel) load instead of a serializing
        # single-row broadcast DMA
        sf = []
        for t in range(T):
            kt = int(KTOT[t])
            sf.append(np.ascontiguousarray(
                srcloc[t, :, 0:kt].T.reshape(kt * 128)))
        sf = np.concatenate(sf)
        sf = np.where(sf > 127, -1, sf).astype(np.int8)     # pad -> -1
        srcflat = np.tile(sf[None, :], (128, 1))            # [128, Σ kt*128]
        ins.append({"slidx": slidx, "srcflat": srcflat})
    foff = np.concatenate([[0], np.cumsum((KTOT * 128).astype(np.int64))])
    return ins, cpt, KL, KH, soff9, foff


def _gcalls(kl, kh):
    """Split a tile's (low, high) chunk blocks into dma_gather calls of at
    most MAXI indices: list of (chunk0, nchunks, is_high)."""
    calls = []
    for base, k, hi in ((0, kl, False), (kl, kh, True)):
        c0 = 0
        while c0 < k:
            n = min(k - c0, MAXI // 128)
            calls.append((base + c0, n, hi))
            c0 += n
    return calls


def _build(cpt, KL, KH, soff, foff):
    nc = bacc.Bacc("TRN2", target_bir_lowering=False, debug=False,
                   num_devices=NCORES, num_swdge_queues=4)

    # ---- external inputs (per core) ----
    d_xT = nc.dram_tensor("xT", [2, 128, NPAD], F16, kind="ExternalInput")
    d_W1 = nc.dram_tensor("W1", [2, 128, F1], F16, kind="ExternalInput")
    d_W1T = nc.dram_tensor("W1T", [F1, FIN], F32, kind="ExternalInput")
    d_Wl1 = nc.dram_tensor("Wl1", [F1, H], F32, kind="ExternalInput")
    d_Wr1 = nc.dram_tensor("Wr1", [F1, H], F32, kind="ExternalInput")
    d_W2 = nc.dram_tensor("W2", [2, 128, C], F16, kind="ExternalInput")
    d_W2T = nc.dram_tensor("W2T", [C, NH], F32, kind="ExternalInput")
    d_Wl2 = nc.dram_tensor("Wl2", [C, 1], F32, kind="ExternalInput")
    d_Wr2 = nc.dram_tensor("Wr2", [C, 1], F32, kind="ExternalInput")
    d_b1 = nc.dram_tensor("b1f", [128, NH], F32, kind="ExternalInput")
    d_b2 = nc.dram_tensor("b2f", [128, C], F32, kind="ExternalInput")
    d_iota = nc.dram_tensor("iota", [128, 128], F16, kind="ExternalInput")
    d_slidx = nc.dram_tensor("slidx", [128, int(soff[-1])], I16,
                             kind="ExternalInput")
    d_srcflat = nc.dram_tensor("srcflat", [128, int(foff[-1])], I8,
                               kind="ExternalInput")
    d_iotac = nc.dram_tensor("iotac", [128, 1], F32, kind="ExternalInput")

    d_out = nc.dram_tensor("out", [NPC, C], F32, kind="ExternalOutput")

    # ---- internal DRAM tables ----
    d_t1loc = nc.dram_tensor("t1loc", [NPC, 40], F16)       # [h | er] compact
    d_el1loc = nc.dram_tensor("el1loc", [NPAD, H], F16)
    d_t1c = nc.dram_tensor("t1c", [N, 40], F16, addr_space="Shared")
    d_t1 = nc.dram_tensor("t1", [N, ROW], F16)               # 256B gather rows
    d_t2loc = nc.dram_tensor("t2loc", [NPC, 48], F16)       # [h2 | er2]
    d_el2loc = nc.dram_tensor("el2loc", [NPAD, 1], F16)
    d_t2c = nc.dram_tensor("t2c", [N, 48], F16, addr_space="Shared")
    d_t2 = nc.dram_tensor("t2", [N, ROW], F16)

    groups = [list(range(NCORES))]

    def expand_table(wp, d_comp, d_tab, width):
        """Re-layout [N, width] f16 -> leading cols of [N, ROW] f16 rows.
        Staged through SBUF tiles holding full 256B rows so both DMA sides
        use large contiguous descriptors (load: 128 compact rows per
        descriptor; store: 32KB full-row runs). Pad cols carry SBUF garbage —
        they are never read downstream. Tail (80 rows) handled separately."""
        CH = 8
        full = N // 128                    # 390 full tiles
        tail = N - full * 128              # 80 rows
        per = (full + CH - 1) // CH
        for i in range(CH):
            t0 = i * per
            n = min(per, full - t0)
            stc = wp.tile([128, per * 48], F16, tag="expc")
            nc.sync.dma_start(
                out=stc[:, 0:n * width].rearrange("p (i w) -> p i w", w=width),
                in_=d_comp.ap()[128 * t0:128 * (t0 + n), :]
                .rearrange("(i p) w -> p i w", p=128))
            st = wp.tile([128, per, ROW], F16, tag="exp")
            nc.vector.tensor_copy(
                out=st[:, 0:n, 0:width],
                in_=stc[:, 0:n * width].rearrange("p (i w) -> p i w", w=width))
            nc.sync.dma_start(
                out=d_tab.ap()[128 * t0:128 * (t0 + n), :]
                .rearrange("(i p) w -> p i w", p=128),
                in_=st[:, 0:n, :])
        st = wp.tile([128, 1, ROW], F16, tag="exp")
        nc.sync.dma_start(out=st[0:tail, 0, 0:width],
                          in_=d_comp.ap()[128 * full:N, :])
        nc.sync.dma_start(out=d_tab.ap()[128 * full:N, :],
                          in_=st[0:tail, 0, :])

    with tile.TileContext(nc, num_cores=NCORES) as tc:
        with (
            tc.tile_pool(name="const", bufs=1) as cpool,
            tc.tile_pool(name="rt", bufs=1) as rtpool,
            tc.tile_pool(name="work", bufs=2) as wp,
            tc.tile_pool(name="gath", bufs=3) as gp,
            tc.tile_pool(name="mask", bufs=3) as mp,
            tc.tile_pool(name="small", bufs=3) as sp,
            tc.tile_pool(name="psA", bufs=2, space="PSUM") as psA,   # agg matmuls
            tc.tile_pool(name="psD", bufs=2, space="PSUM") as psD,   # dense matmuls
            tc.tile_pool(name="psT", bufs=2, space="PSUM") as psT,
            tc.tile_pool(name="psE", bufs=2, space="PSUM") as psE,   # transposes
        ):
            nc.gpsimd.load_library(library_config.mlp)
            # ---------- constants ----------
            iota = cpool.tile([128, 128], F16)
            nc.sync.dma_start(out=iota[:], in_=d_iota.ap())
            b1sb = cpool.tile([128, NH], F32)
            nc.sync.dma_start(out=b1sb[:], in_=d_b1.ap())
            b2sb = cpool.tile([128, C], F32)
            nc.sync.dma_start(out=b2sb[:], in_=d_b2.ap())
            ident = cpool.tile([128, 128], F16)
            make_identity(nc, ident[:])
            iotac = cpool.tile([128, 1], F32)
            nc.sync.dma_start(out=iotac[:], in_=d_iotac.ap())
            iotab = cpool.tile([128, 1], F16)
            nc.vector.tensor_copy(out=iotab[:], in_=iotac[:])
            iotab8 = cpool.tile([128, 1], I8)
            nc.vector.tensor_copy(out=iotab8[:], in_=iotac[:])
            czero = cpool.tile([128, 1], F32)
            nc.vector.memset(czero[:], 0.0)
            ceps = cpool.tile([128, 1], F32)
            nc.vector.memset(ceps[:], 1e-12)
            # concatenated dense rhs: [W1 | B1 | A1] and [W2 | B2 | A2] so each
            # dense pass is a single PSUM accumulation group (start=True clears
            # the whole bank, so groups must not share a bank)
            W1cat = cpool.tile([128, 2, 48], F16)
            nc.sync.dma_start(out=W1cat[:, :, 0:32],
                              in_=d_W1.ap().rearrange("q p f -> p q f"))
            W2cat = cpool.tile([128, 2, 49], F16)
            nc.sync.dma_start(out=W2cat[:, :, 0:47],
                              in_=d_W2.ap().rearrange("q p f -> p q f"))
            W1Tsb = cpool.tile([F1, FIN], F32)
            nc.sync.dma_start(out=W1Tsb[:], in_=d_W1T.ap())
            W2Tsb = cpool.tile([C, NH], F32)
            nc.sync.dma_start(out=W2Tsb[:], in_=d_W2T.ap())
            Wl1sb = cpool.tile([F1, H], F32)
            nc.sync.dma_start(out=Wl1sb[:], in_=d_Wl1.ap())
            Wr1sb = cpool.tile([F1, H], F32)
            nc.sync.dma_start(out=Wr1sb[:], in_=d_Wr1.ap())
            Wl2sb = cpool.tile([C, 1], F32)
            nc.sync.dma_start(out=Wl2sb[:], in_=d_Wl2.ap())
            Wr2sb = cpool.tile([C, 1], F32)
            nc.sync.dma_start(out=Wr2sb[:], in_=d_Wr2.ap())

            # A1/B1 = W1 @ Wl1 / W1 @ Wr1; A2/B2 = W2 @ Wl2 / W2 @ Wr2.
            for q in range(2):
                pa = psD.tile([128, H], F32, tag="dense")
                nc.tensor.matmul(out=pa[:], lhsT=W1Tsb[:, q * 128:(q + 1) * 128],
                                 rhs=Wl1sb[:], start=True, stop=True)
                nc.vector.tensor_copy(out=W1cat[:, q, 40:48], in_=pa[:])
                pb = psD.tile([128, H], F32, tag="dense")
                nc.tensor.matmul(out=pb[:], lhsT=W1Tsb[:, q * 128:(q + 1) * 128],
                                 rhs=Wr1sb[:], start=True, stop=True)
                nc.vector.tensor_copy(out=W1cat[:, q, 32:40], in_=pb[:])
                pc = psD.tile([128, 1], F32, tag="dense")
                nc.tensor.matmul(out=pc[:], lhsT=W2Tsb[:, q * 128:(q + 1) * 128],
                                 rhs=Wl2sb[:], start=True, stop=True)
                nc.vector.tensor_copy(out=W2cat[:, q, 48:49], in_=pc[:])
                pd = psD.tile([128, 1], F32, tag="dense")
                nc.tensor.matmul(out=pd[:], lhsT=W2Tsb[:, q * 128:(q + 1) * 128],
                                 rhs=Wr2sb[:], start=True, stop=True)
                nc.vector.tensor_copy(out=W2cat[:, q, 47:48], in_=pd[:])

            rT = rtpool.tile([128, 2, NPAD], F16)   # transposed post-elu layer-1 out

            # ---------- phase D1: h/el/er for owned nodes ----------
            for t in range(T):
                rows = 128 if t < T - 1 else LAST_ROWS
                xa = wp.tile([128, 2, 128], F16, tag="xa")
                nc.sync.dma_start(
                    out=xa[:], in_=d_xT.ap()[:, :, t * 128:(t + 1) * 128]
                    .rearrange("q p n -> p q n"))
                ps = psD.tile([128, 48], F32, tag="dense")
                for q in range(2):
                    nc.tensor.matmul(out=ps[:], lhsT=xa[:, q, :], rhs=W1cat[:, q, :],
                                     start=q == 0, stop=q == 1)
                hsb = wp.tile([128, 48], F16, tag="hsb")
                nc.scalar.copy(out=hsb[:], in_=ps[:])
                nc.scalar.dma_start(out=d_t1loc.ap()[t * 128:t * 128 + rows, :],
                                    in_=hsb[0:rows, 0:40])
                nc.scalar.dma_start(out=d_el1loc.ap()[t * 128:(t + 1) * 128, :],
                                    in_=hsb[:, 40:48])

            # ---------- C1: share + expand layer-1 table ----------
            nc.gpsimd.collective_compute(
                "AllGather", ALU.bypass, replica_groups=groups,
                ins=[d_t1loc.ap()], outs=[d_t1c.ap()])
            expand_table(wp, d_t1c, d_t1, 40)

            # ---------- phase S1 (+ fused D2) ----------
            for t in range(T):
                rows = 128 if t < T - 1 else LAST_ROWS
                kl, kh = int(KL[t]), int(KH[t])
                kt = kl + kh
                pk = sp.tile([128, cpt * 9], I16, tag="pk")
                nc.sync.dma_start(out=pk[:, 0:kt * 9],
                                  in_=d_slidx.ap()[:, int(soff[t]):int(soff[t + 1])])
                slv = pk[:, 0:kt].bitcast(F16)

                G1 = gp.tile([128, cpt, ROW], F16, tag="G1")
                for qi, (c0, ck, hi) in enumerate(_gcalls(kl, kh)):
                    src = d_t1.ap()[SPLIT:, :] if hi else d_t1.ap()
                    nc.gpsimd.dma_gather(
                        G1[:, c0:c0 + ck, :], src,
                        pk[:, kt + c0 * 8:kt + (c0 + ck) * 8],
                        ck * 128, ck * 128, ROW,
                        queue_num=(t * 3 + qi) % 4, single_packet=True)

                # transposed mask maskT[p, k*128+j] = (srcflat[k*128+j] == p);
                # el per edge = maskT.T @ el_tile (replaces per-edge el gathers)
                slb = mp.tile([128, cpt * 128], I8, tag="slb")
                nc.sync.dma_start(
                    out=slb[:, 0:kt * 128],
                    in_=d_srcflat.ap()[:, int(foff[t]):int(foff[t + 1])])
                mskT = mp.tile([128, cpt * 128], F16, tag="mskT")
                nc.vector.tensor_tensor(
                    out=mskT[:, 0:kt * 128], in0=slb[:, 0:kt * 128],
                    in1=iotab8[:].to_broadcast([128, kt * 128]),
                    op=ALU.is_equal)
                elt = sp.tile([128, H], F16, tag="elt")
                nc.scalar.dma_start(out=elt[:],
                                    in_=d_el1loc.ap()[t * 128:(t + 1) * 128, :])
                pse = psE.tile([128, cpt * H], F32, tag="elexp")
                for k in range(kt):
                    nc.tensor.matmul(out=pse[:, k * H:(k + 1) * H],
                                     lhsT=mskT[:, k * 128:(k + 1) * 128], rhs=elt[:],
                                     start=k == 0, stop=k == kt - 1,
                                     skip_group_check=True)

                s = wp.tile([128, cpt, H], F32, tag="s")
                nc.vector.tensor_tensor(
                    out=s[:, 0:kt, :], in0=G1[:, 0:kt, 32:40],
                    in1=pse[:, 0:kt * H].rearrange("p (k h) -> p k h", h=H),
                    op=ALU.add)
                sL = wp.tile([128, cpt, H], F32, tag="sL")
                nc.vector.scalar_tensor_tensor(
                    out=sL[:, 0:kt, :], in0=s[:, 0:kt, :], scalar=0.2,
                    in1=s[:, 0:kt, :], op0=ALU.mult, op1=ALU.max)
                rhs = wp.tile([128, cpt, 264], F16, tag="rhs")
                nc.scalar.activation(out=rhs[:, 0:kt, 256:264], in_=sL[:, 0:kt, :],
                                     func=ACT.Exp)

                msk = wp.tile([128, cpt, 128], F16, tag="msk")
                nc.vector.tensor_tensor(
                    out=msk[:, 0:kt, :],
                    in0=iota[:].rearrange("p (o j) -> p o j", o=1)
                        .to_broadcast([128, kt, 128]),
                    in1=slv.rearrange("p (k o) -> p k o", o=1)
                        .to_broadcast([128, kt, 128]),
                    op=ALU.is_equal)
                nc.vector.tensor_tensor(
                    out=rhs[:, 0:kt, 0:256].rearrange("p k (h f) -> p k h f", h=H),
                    in0=rhs[:, 0:kt, 256:264].rearrange("p k (h o) -> p k h o", o=1)
                        .to_broadcast([128, kt, H, F1]),
                    in1=G1[:, 0:kt, 0:32].rearrange("p k (o f) -> p k o f", o=1)
                        .to_broadcast([128, kt, H, F1]),
                    op=ALU.mult)

                ps1 = psA.tile([128, 264], F32, tag="agg")
                for k in range(kt):
                    nc.tensor.matmul(out=ps1[:], lhsT=msk[:, k, :], rhs=rhs[:, k, :],
                                     start=k == 0, stop=k == kt - 1)

                # epilogue: out1 = agg/denom + b1 ; r = elu(out1); rT = r.T
                dn = sp.tile([128, H], F32, tag="dn")
                nc.vector.tensor_tensor(out=dn[:], in0=ps1[:, 256:264],
                                        in1=ceps[:].to_broadcast([128, H]),
                                        op=ALU.max)
                rc = sp.tile([128, H], F32, tag="rc")
                nc.vector.reciprocal(out=rc[:], in_=dn[:])
                o1 = wp.tile([128, NH], F32, tag="o1")
                nc.vector.tensor_tensor(
                    out=o1[:].rearrange("p (h f) -> p h f", h=H),
                    in0=ps1[:, 0:256].rearrange("p (h f) -> p h f", h=H),
                    in1=rc[:].rearrange("p (h o) -> p h o", o=1)
                        .to_broadcast([128, H, F1]),
                    op=ALU.mult)
                o1b = wp.tile([128, NH], F32, tag="o1b")
                nc.vector.tensor_tensor(out=o1b[:], in0=o1[:], in1=b1sb[:], op=ALU.add)
                p_ = wp.tile([128, NH], F32, tag="p_")
                nc.vector.tensor_tensor(out=p_[:], in0=o1b[:],
                                        in1=czero[:].to_broadcast([128, NH]),
                                        op=ALU.max)
                q_ = wp.tile([128, NH], F32, tag="q_")
                nc.vector.tensor_tensor(out=q_[:], in0=o1b[:],
                                        in1=czero[:].to_broadcast([128, NH]),
                                        op=ALU.min)
                eq = wp.tile([128, NH], F32, tag="eq")
                nc.scalar.activation(out=eq[:], in_=q_[:], func=ACT.Exp)
                r_ = wp.tile([128, NH], F16, tag="r_")
                nc.vector.scalar_tensor_tensor(out=r_[:], in0=eq[:], scalar=-1.0,
                                               in1=p_[:], op0=ALU.add, op1=ALU.add)
                for q in range(2):
                    pt = psT.tile([128, 128], F16, tag="pt")
                    nc.tensor.transpose(out=pt[:], in_=r_[:, q * 128:(q + 1) * 128],
                                        identity=ident[:])
                    nc.scalar.copy(out=rT[:, q, t * 128:(t + 1) * 128], in_=pt[:])

                # D2: h2/el2/er2 for this tile
                ps2 = psD.tile([128, 49], F32, tag="dense")
                for q in range(2):
                    nc.tensor.matmul(out=ps2[:], lhsT=rT[:, q, t * 128:(t + 1) * 128],
                                     rhs=W2cat[:, q, :], start=q == 0, stop=q == 1)
                h2sb = wp.tile([128, 49], F16, tag="h2sb")
                nc.scalar.copy(out=h2sb[:], in_=ps2[:])
                nc.scalar.dma_start(out=d_t2loc.ap()[t * 128:t * 128 + rows, :],
                                    in_=h2sb[0:rows, 0:48])
                nc.scalar.dma_start(out=d_el2loc.ap()[t * 128:(t + 1) * 128, :],
                                    in_=h2sb[:, 48:49])

            # ---------- C2: share + expand layer-2 table ----------
            nc.gpsimd.collective_compute(
                "AllGather", ALU.bypass, replica_groups=groups,
                ins=[d_t2loc.ap()], outs=[d_t2c.ap()])
            expand_table(wp, d_t2c, d_t2, 48)

            # ---------- phase S2 ----------
            for t in range(T):
                rows = 128 if t < T - 1 else LAST_ROWS
                kl, kh = int(KL[t]), int(KH[t])
                kt = kl + kh
                pk = sp.tile([128, cpt * 9], I16, tag="pk")
                nc.sync.dma_start(out=pk[:, 0:kt * 9],
                                  in_=d_slidx.ap()[:, int(soff[t]):int(soff[t + 1])])
                slv = pk[:, 0:kt].bitcast(F16)

                Gt = gp.tile([128, cpt, ROW], F16, tag="Gt")
                for qi, (c0, ck, hi) in enumerate(_gcalls(kl, kh)):
                    src = d_t2.ap()[SPLIT:, :] if hi else d_t2.ap()
                    nc.gpsimd.dma_gather(
                        Gt[:, c0:c0 + ck, :], src,
                        pk[:, kt + c0 * 8:kt + (c0 + ck) * 8],
                        ck * 128, ck * 128, ROW,
                        queue_num=(t * 3 + qi) % 4, single_packet=True)

                slb = mp.tile([128, cpt * 128], I8, tag="slb")
                nc.sync.dma_start(
                    out=slb[:, 0:kt * 128],
                    in_=d_srcflat.ap()[:, int(foff[t]):int(foff[t + 1])])
                mskT = mp.tile([128, cpt * 128], F16, tag="mskT")
                nc.vector.tensor_tensor(
                    out=mskT[:, 0:kt * 128], in0=slb[:, 0:kt * 128],
                    in1=iotab8[:].to_broadcast([128, kt * 128]),
                    op=ALU.is_equal)
                el2t = sp.tile([128, 1], F16, tag="el2t")
                nc.scalar.dma_start(out=el2t[:],
                                    in_=d_el2loc.ap()[t * 128:(t + 1) * 128, :])
                pse2 = psE.tile([128, cpt], F32, tag="elexp")
                for k in range(kt):
                    nc.tensor.matmul(out=pse2[:, k:k + 1],
                                     lhsT=mskT[:, k * 128:(k + 1) * 128], rhs=el2t[:],
                                     start=k == 0, stop=k == kt - 1,
                                     skip_group_check=True)

                s2 = sp.tile([128, cpt], F32, tag="s2")
                nc.vector.tensor_tensor(
                    out=s2[:, 0:kt],
                    in0=Gt[:, 0:kt, 47:48].rearrange("p k o -> p (k o)"),
                    in1=pse2[:, 0:kt], op=ALU.add)
                s2s = sp.tile([128, cpt], F32, tag="s2s")
                nc.vector.tensor_scalar(out=s2s[:, 0:kt], in0=s2[:, 0:kt],
                                        scalar1=0.2, scalar2=None, op0=ALU.mult)
                sL2 = sp.tile([128, cpt], F32, tag="sL2")
                nc.vector.tensor_tensor(out=sL2[:, 0:kt], in0=s2s[:, 0:kt],
                                        in1=s2[:, 0:kt], op=ALU.max)
                e2f = sp.tile([128, cpt], F32, tag="e2f")
                nc.scalar.activation(out=e2f[:, 0:kt], in_=sL2[:, 0:kt],
                                     func=ACT.Exp)

                rhs2 = wp.tile([128, cpt, 48], F16, tag="rhs2")
                nc.scalar.copy(
                    out=rhs2[:, 0:kt, 47:48].rearrange("p k o -> p (k o)"),
                    in_=e2f[:, 0:kt])
                nc.vector.tensor_tensor(
                    out=rhs2[:, 0:kt, 0:47], in0=Gt[:, 0:kt, 0:47],
                    in1=e2f[:, 0:kt].rearrange("p (k o) -> p k o", o=1)
                        .to_broadcast([128, kt, C]),
                    op=ALU.mult)
                msk = wp.tile([128, cpt, 128], F16, tag="msk")
                nc.vector.tensor_tensor(
                    out=msk[:, 0:kt, :],
                    in0=iota[:].rearrange("p (o j) -> p o j", o=1)
                        .to_broadcast([128, kt, 128]),
                    in1=slv.rearrange("p (k o) -> p k o", o=1)
                        .to_broadcast([128, kt, 128]),
                    op=ALU.is_equal)
                ps3 = psA.tile([128, 48], F32, tag="agg")
                for k in range(kt):
                    nc.tensor.matmul(out=ps3[:], lhsT=msk[:, k, :], rhs=rhs2[:, k, :],
                                     start=k == 0, stop=k == kt - 1)

                # epilogue: out2 = agg2/denom2 + b2, then log_softmax
                dn2 = sp.tile([128, 1], F32, tag="dn2")
                nc.vector.tensor_tensor(out=dn2[:], in0=ps3[:, 47:48],
                                        in1=ceps[:], op=ALU.max)
                rc2 = sp.tile([128, 1], F32, tag="rc2")
                nc.vector.reciprocal(out=rc2[:], in_=dn2[:])
                o2b = wp.tile([128, C], F32, tag="o2b")
                nc.vector.scalar_tensor_tensor(out=o2b[:], in0=ps3[:, 0:47],
                                               scalar=rc2[:, 0:1], in1=b2sb[:],
                                               op0=ALU.mult, op1=ALU.add)
                mx = sp.tile([128, 1], F32, tag="mx")
                nc.vector.tensor_reduce(out=mx[:], in_=o2b[:],
                                        axis=mybir.AxisListType.X, op=ALU.max)
                xm = wp.tile([128, C], F32, tag="xm")
                nc.vector.tensor_tensor(out=xm[:], in0=o2b[:],
                                        in1=mx[:, 0:1].to_broadcast([128, C]),
                                        op=ALU.subtract)
                ex = wp.tile([128, C], F32, tag="ex")
                se = sp.tile([128, 1], F32, tag="se")
                nc.scalar.activation(out=ex[:], in_=xm[:], func=ACT.Exp,
                                     accum_out=se[:])
                ls = sp.tile([128, 1], F32, tag="ls")
                nc.scalar.activation(out=ls[:], in_=se[:], func=ACT.Ln)
                fin = wp.tile([128, C], F32, tag="fin")
                nc.vector.tensor_tensor(out=fin[:], in0=xm[:],
                                        in1=ls[:, 0:1].to_broadcast([128, C]),
                                        op=ALU.subtract)
                nc.scalar.dma_start(out=d_out.ap()[t * 128:t * 128 + rows, :],
                                    in_=fin[0:rows, :])

    nc.compile()
    return nc


def _make_inputs(x, edge_src, edge_dst, W1, Wl1, Wr1, b1, W2, Wl2, Wr2, b2):
    edge_ins, cpt, KL, KH, soff, foff = _preprocess(x, edge_src, edge_dst)
    x = np.asarray(x, dtype=np.float32)
    W1 = np.asarray(W1, dtype=np.float32)
    W2 = np.asarray(W2, dtype=np.float32)
    iota = np.tile(np.arange(128, dtype=np.float16), (128, 1))
    b1f = np.tile(np.tile(np.asarray(b1, np.float32), H)[None, :], (128, 1))
    b2f = np.tile(np.asarray(b2, np.float32)[None, :], (128, 1))
    common = {
        "W1": W1.reshape(2, 128, F1).astype(np.float16),
        "W1T": np.ascontiguousarray(W1.T),
        "Wl1": np.asarray(Wl1, np.float32),
        "Wr1": np.asarray(Wr1, np.float32),
        "W2": W2.reshape(2, 128, C).astype(np.float16),
        "W2T": np.ascontiguousarray(W2.T),
        "Wl2": np.asarray(Wl2, np.float32),
        "Wr2": np.asarray(Wr2, np.float32),
        "b1f": b1f, "b2f": b2f, "iota": iota,
        "iotac": np.arange(128, dtype=np.float32)[:, None],
    }
    in_maps = []
    for c in range(NCORES):
        xT = np.zeros((2, 128, NPAD), dtype=np.float16)
        xs = np.ascontiguousarray(x[c * NPC:(c + 1) * NPC].T)   # [256, NPC]
        xT[:, :, :NPC] = xs.reshape(2, 128, NPC)
        m = dict(common)
        m["xT"] = xT
        m.update(edge_ins[c])
        in_maps.append(m)
    return in_maps, (cpt, tuple(KL), tuple(KH), tuple(soff), tuple(foff))


def _run(inputs, trace=False):
    in_maps, key = _make_inputs(**inputs)
    if key not in _cache:
        cpt, KL, KH, soff, foff = key
        _cache[key] = _build(cpt, np.array(KL), np.array(KH),
                             np.array(soff), np.array(foff))
    nc = _cache[key]
    bkr = run_bass_kernel_spmd(nc, in_maps, list(range(NCORES)), trace=trace)
    out = np.concatenate([bkr.results[c]["out"] for c in range(NCORES)], axis=0)
    return out.astype(np.float32), bkr


def kernel(**inputs):
    out, _ = _run(inputs, trace=False)
    return out
